# revision 30
# baseline (speedup 1.0000x reference)
import sys
from contextlib import ExitStack

import numpy as np

sys.path.insert(0, "/opt/trn_rl_repo")

import jax

# Persistent compilation cache: warm calls skip the per-call NEFF/walrus
# recompile inside the neuronx_cc hook (the executable is cached on disk
# keyed by HLO, which is identical across calls).
try:
    jax.config.update("jax_compilation_cache_dir", "/tmp/bass_jax_cache")
    jax.config.update("jax_persistent_cache_min_compile_time_secs", 0.0)
    jax.config.update("jax_persistent_cache_min_entry_size_bytes", 0)
except Exception:
    pass

import concourse.bass as bass
import concourse.tile as tile
from concourse import bacc, mybir
from concourse.bass_utils import run_bass_kernel_spmd

# Problem constants (hardcoded per harness contract)
N = 10000
D_IN = 12
E = N * D_IN            # 120000 edges
T = E * D_IN            # 1440000 triplets
K_R = 16
K_A = 8
HID = 64
OUT_D = 32
IN_DIM = 2 * K_R + K_A  # 40
N24 = K_R + K_A         # 24 per-triplet (dik/cos) features
GAMMA = 8.0             # same gamma for radial and angular RBFs
EPS = 1e-8
POISON = 30.0           # exp(-8*(30-c)^2) == 0 in f32; fits fp16

NCORES = 8
TD = T // NCORES        # 180000 triplets per core
ED = E // NCORES        # 15000 edges per core
TT = 504                # triplets per tile = 42 edges * 12

# params packing offsets (flat f32 tensor)
P_C16 = 0               # [16,1] rc
P_KAK = 16              # [1,24] 2*g*rc on dik features, else 0
P_KBK = 40              # [1,24] -g on dik features, else 0
P_KAC = 64              # [1,24] (2*g/127)*ac on cos features, else 0
P_KBC = 88              # [1,24] -g/(127*127) on cos features, else 0
P_B24 = 112             # [24,1] -g*c^2
P_W1A = 136             # [16,64] W1 rows 0..16 (dij features)
P_W1B = 1160            # [24,64] W1 rows 16..40
P_B1 = 2696             # [64,1]
P_W2 = 2760             # [64,32]
P_TOT = 4808

F32 = mybir.dt.float32
F16 = mybir.dt.float16
I8 = mybir.dt.int8

_PROG = None
LAST_RESULTS = None
LAST_RUN_S = None


def _build_program():
    nc = bacc.Bacc(
        "TRN2", target_bir_lowering=False, debug=False, num_devices=NCORES
    )
    # xd: per-edge distance (fp16, unpoisoned); the dij RBF block is
    # per-edge and broadcast over the 12 triplets of each edge on device.
    XD = nc.dram_tensor("xd", [1, ED], F16, kind="ExternalInput").ap()
    # xk: dik per triplet (fp16, poisoned where k==j)
    XK = nc.dram_tensor("xk", [1, TD], F16, kind="ExternalInput").ap()
    # xc: cos per triplet quantized to int8 (enc = round(cos*127); the
    # decode scale 1/127 is folded into the kaC/kbC stationaries)
    XC = nc.dram_tensor("xc", [1, TD], I8, kind="ExternalInput").ap()
    PRM = nc.dram_tensor("params", [P_TOT], F32, kind="ExternalInput").ap()
    Y = nc.dram_tensor("y", [OUT_D, ED], F16, kind="ExternalOutput").ap()

    with tile.TileContext(nc) as tc, ExitStack() as ctx:
        consts = ctx.enter_context(tc.tile_pool(name="consts", bufs=1))
        inp = ctx.enter_context(tc.tile_pool(name="inp", bufs=4))
        mid = ctx.enter_context(tc.tile_pool(name="mid", bufs=3))
        hp = ctx.enter_context(tc.tile_pool(name="hp", bufs=3))
        psa = ctx.enter_context(
            tc.tile_pool(name="psa", bufs=2, space=bass.MemorySpace.PSUM)
        )
        ps0 = ctx.enter_context(
            tc.tile_pool(name="ps0", bufs=2, space=bass.MemorySpace.PSUM)
        )
        ps1 = ctx.enter_context(
            tc.tile_pool(name="ps1", bufs=2, space=bass.MemorySpace.PSUM)
        )
        ps2 = ctx.enter_context(
            tc.tile_pool(name="ps2", bufs=2, space=bass.MemorySpace.PSUM)
        )

        c16t = consts.tile([K_R, 1], F32)
        nc.gpsimd.dma_start(
            c16t[:], PRM[P_C16 : P_C16 + 16].rearrange("(p f) -> p f", p=16)
        )
        kak = consts.tile([1, N24], F32)
        nc.gpsimd.dma_start(kak[:], PRM[P_KAK : P_KAK + 24].unsqueeze(0))
        kbk = consts.tile([1, N24], F32)
        nc.gpsimd.dma_start(kbk[:], PRM[P_KBK : P_KBK + 24].unsqueeze(0))
        kac = consts.tile([1, N24], F32)
        nc.gpsimd.dma_start(kac[:], PRM[P_KAC : P_KAC + 24].unsqueeze(0))
        kbc = consts.tile([1, N24], F32)
        nc.gpsimd.dma_start(kbc[:], PRM[P_KBC : P_KBC + 24].unsqueeze(0))
        b24t = consts.tile([N24, 1], F32)
        nc.gpsimd.dma_start(
            b24t[:], PRM[P_B24 : P_B24 + 24].rearrange("(p f) -> p f", p=24)
        )
        w1at = consts.tile([K_R, HID], F32)
        nc.gpsimd.dma_start(
            w1at[:], PRM[P_W1A : P_W1A + 1024].rearrange("(p f) -> p f", p=16)
        )
        w1bt = consts.tile([N24, HID], F32)
        nc.gpsimd.dma_start(
            w1bt[:], PRM[P_W1B : P_W1B + 1536].rearrange("(p f) -> p f", p=24)
        )
        b1t = consts.tile([HID, 1], F32)
        nc.gpsimd.dma_start(
            b1t[:], PRM[P_B1 : P_B1 + 64].rearrange("(p f) -> p f", p=64)
        )
        w2t = consts.tile([HID, OUT_D], F32)
        nc.gpsimd.dma_start(
            w2t[:], PRM[P_W2 : P_W2 + 2048].rearrange("(p f) -> p f", p=64)
        )
        out_sb = consts.tile([OUT_D, ED], F32)
        out16 = consts.tile([OUT_D, ED], F16)

        G = TT // D_IN  # edges per tile

        def emit_tile(t0, e0, tt, g):
            """One tile of `tt` triplets / `g` edges; t0/e0 may be symbolic."""
            # --- per-edge dij RBF block -> W1a contribution [HID, g] ---
            dbc = inp.tile([K_R, g], F16)
            nc.gpsimd.dma_start(
                dbc[:], XD[:, bass.ds(e0, g)].partition_broadcast(K_R)
            )
            dsub = mid.tile([K_R, g], F32)
            nc.vector.tensor_scalar_sub(dsub[:], dbc[:], c16t[:])
            dsq = mid.tile([K_R, g], F32)
            nc.vector.tensor_mul(dsq[:], dsub[:], dsub[:])
            fij = mid.tile([K_R, g], F32)
            nc.scalar.activation(
                fij[:], dsq[:], mybir.ActivationFunctionType.Exp, scale=-GAMMA
            )
            pa = psa.tile([HID, g], F32)
            nc.tensor.matmul(pa[:], w1at[:], fij[:])
            ha = hp.tile([HID, g], F32)
            nc.scalar.copy(ha[:], pa[:])

            # --- per-triplet dik/cos features -> W1b contribution [HID, tt] ---
            xkt = inp.tile([1, tt], F16)
            nc.gpsimd.dma_start(xkt[:], XK[:, bass.ds(t0, tt)])
            xct = inp.tile([1, tt], I8)
            nc.gpsimd.dma_start(xct[:], XC[:, bass.ds(t0, tt)])
            xkf = mid.tile([1, tt], F32)
            nc.vector.tensor_copy(xkf[:], xkt[:])
            xk2 = mid.tile([1, tt], F32)
            nc.vector.tensor_mul(xk2[:], xkt[:], xkt[:])
            xcf = mid.tile([1, tt], F32)
            nc.vector.tensor_copy(xcf[:], xct[:])
            xc2 = mid.tile([1, tt], F32)
            nc.vector.tensor_mul(xc2[:], xcf[:], xcf[:])
            p0 = ps0.tile([N24, tt], F32)
            nc.tensor.matmul(p0[:], kak[:], xkf[:], start=True, stop=False)
            nc.tensor.matmul(p0[:], kbk[:], xk2[:], start=False, stop=False)
            nc.tensor.matmul(p0[:], kac[:], xcf[:], start=False, stop=False)
            nc.tensor.matmul(p0[:], kbc[:], xc2[:], start=False, stop=True)
            ft2 = mid.tile([N24, tt], F32)
            nc.scalar.activation(
                ft2[:], p0[:], mybir.ActivationFunctionType.Exp, bias=b24t[:]
            )
            p1 = ps1.tile([HID, tt], F32)
            nc.tensor.matmul(p1[:], w1bt[:], ft2[:])

            # --- combine (broadcast per-edge term over 12 triplets) + MLP ---
            hs = hp.tile([HID, tt], F32)
            nc.vector.tensor_add(
                hs[:].rearrange("p (g s) -> p g s", s=D_IN),
                p1[:].rearrange("p (g s) -> p g s", s=D_IN),
                ha[:].unsqueeze(2).broadcast_to([HID, g, D_IN]),
            )
            h = hp.tile([HID, tt], F32)
            nc.scalar.activation(
                h[:], hs[:], mybir.ActivationFunctionType.Silu, bias=b1t[:]
            )
            p2 = ps2.tile([OUT_D, tt], F32)
            nc.tensor.matmul(p2[:], w2t[:], h[:])

            nc.vector.tensor_reduce(
                out_sb[:, bass.ds(e0, g)],
                p2[:].rearrange("p (g s) -> p g s", s=D_IN),
                axis=mybir.AxisListType.X,
                op=mybir.AluOpType.add,
            )

        nt_full = TD // TT
        tc.For_i_unrolled(
            0,
            nt_full,
            1,
            lambda iv: emit_tile(iv * TT, iv * (TT // D_IN), TT, TT // D_IN),
            max_unroll=8,
        )
        rem = TD - nt_full * TT
        if rem:
            emit_tile(nt_full * TT, nt_full * G, rem, rem // D_IN)

        nc.scalar.copy(out16[:], out_sb[:])
        nc.gpsimd.dma_start(Y[:], out16[:])

    nc.compile()
    return nc


def _get_program():
    global _PROG
    if _PROG is None:
        _PROG = _build_program()
    return _PROG


def _numpy_fallback(pos, W1, b1, W2, b2, rc, ac, e_e, i_e, j_e, k_e):
    rij = pos[j_e] - pos[i_e]
    rik = pos[k_e] - pos[i_e]
    dij = np.sqrt((rij * rij).sum(-1))
    dik = np.sqrt((rik * rik).sum(-1))
    cos = np.clip((rij * rik).sum(-1) / (dij * dik + EPS), -1.0, 1.0)
    feat = np.concatenate(
        [
            np.exp(-GAMMA * (dij[:, None] - rc[None, :]) ** 2),
            np.exp(-GAMMA * (dik[:, None] - rc[None, :]) ** 2),
            np.exp(-GAMMA * (cos[:, None] - ac[None, :]) ** 2),
        ],
        axis=-1,
    ).astype(np.float32)
    hpre = feat @ W1 + b1
    h = hpre / (1.0 + np.exp(-hpre))
    emb = h @ W2 + b2
    emb *= (k_e != j_e)[:, None].astype(np.float32)
    out = np.zeros((E, OUT_D), np.float32)
    np.add.at(out, e_e, emb)
    return out


def kernel(**inputs) -> np.ndarray:
    global LAST_RESULTS
    pos = np.asarray(inputs["pos"], np.float32)
    W1 = np.asarray(inputs["W1"], np.float32)
    b1 = np.asarray(inputs["b1"], np.float32)
    W2 = np.asarray(inputs["W2"], np.float32)
    b2 = np.asarray(inputs["b2"], np.float32)
    rc = np.asarray(inputs["r_centers"], np.float32)
    ac = np.asarray(inputs["a_centers"], np.float32)
    e_e = np.asarray(inputs["e_e"])
    i_e = np.asarray(inputs["i_e"])
    j_e = np.asarray(inputs["j_e"])
    k_e = np.asarray(inputs["k_e"])

    row = i_e[::D_IN].astype(np.int64)          # source node of each edge
    kidx = (row[:, None] * D_IN + np.arange(D_IN)[None, :]).reshape(-1)  # [T]
    structured = (
        np.array_equal(
            e_e, np.repeat(np.arange(E, dtype=np.int64), D_IN).astype(e_e.dtype)
        )
        and np.array_equal(j_e.astype(np.int64), e_e.astype(np.int64) // D_IN)
        and np.array_equal(i_e.astype(np.int64), np.repeat(row, D_IN))
        and np.array_equal(k_e.astype(np.int64), row[kidx])
    )
    if not structured:
        return _numpy_fallback(pos, W1, b1, W2, b2, rc, ac, e_e, i_e, j_e, k_e)

    # Per-edge geometry on host (E values instead of T), then expand to
    # triplets; device handles RBF + MLP + segment sum.
    col = np.repeat(np.arange(N, dtype=np.int64), D_IN)
    dvec = pos[col] - pos[row]                  # [E,3]
    d = np.sqrt((dvec * dvec).sum(-1))          # [E]
    u = dvec / np.maximum(d, 1e-30)[:, None]    # [E,3] unit vectors

    dik = d[kidx]                               # [T]
    # edge kidx points k->i, so rik = pos[k]-pos[i] = -dvec[kidx]
    cos = np.clip(
        -np.einsum("ts,ts->t", np.repeat(u, D_IN, axis=0), u[kidx]), -1.0, 1.0
    )
    mask = k_e != j_e

    xd = d.astype(np.float16).reshape(1, E)
    xk = np.where(mask, dik, POISON).astype(np.float16).reshape(1, T)
    # masked triplets get enc=0; their (constant) cos-feature contribution is
    # subtracted on the host below
    xc = np.where(mask, np.round(cos * 127.0), 0.0).astype(np.int8).reshape(1, T)

    # dik/cos features: exp(-g*(x-c)^2) = exp(-g*x^2 + 2*g*c*x - g*c^2);
    # for cos the int8 decode scale 1/127 is folded into the coefficients.
    cf24 = np.concatenate([rc, ac]).astype(np.float32)           # [24]
    prm = np.zeros(P_TOT, np.float32)
    prm[P_C16 : P_C16 + 16] = rc
    prm[P_KAK : P_KAK + K_R] = 2.0 * GAMMA * rc
    prm[P_KBK : P_KBK + K_R] = -GAMMA
    prm[P_KAC + K_R : P_KAC + 24] = (2.0 * GAMMA / 127.0) * ac
    prm[P_KBC + K_R : P_KBC + 24] = -GAMMA / (127.0 * 127.0)
    prm[P_B24 : P_B24 + 24] = -GAMMA * cf24 * cf24
    prm[P_W1A : P_W1A + 1024] = W1[:K_R].reshape(-1)
    prm[P_W1B : P_W1B + 1536] = W1[K_R:].reshape(-1)
    prm[P_B1 : P_B1 + 64] = b1
    prm[P_W2 : P_W2 + 2048] = W2.reshape(-1)

    in_maps = []
    for dev in range(NCORES):
        in_maps.append(
            {
                "xd": np.ascontiguousarray(xd[:, dev * ED : (dev + 1) * ED]),
                "xk": np.ascontiguousarray(xk[:, dev * TD : (dev + 1) * TD]),
                "xc": np.ascontiguousarray(xc[:, dev * TD : (dev + 1) * TD]),
                "params": prm,
            }
        )

    import time as _time

    global LAST_RUN_S
    _t0 = _time.time()
    try:
        res = run_bass_kernel_spmd(_get_program(), in_maps, list(range(NCORES)))
    except Exception:
        # transient device errors (NRT_EXEC_UNIT_UNRECOVERABLE) recover on
        # retry; if not, fall back to the (slow but correct) host path
        try:
            res = run_bass_kernel_spmd(
                _get_program(), in_maps, list(range(NCORES))
            )
        except Exception:
            LAST_RUN_S = _time.time() - _t0
            return _numpy_fallback(
                pos, W1, b1, W2, b2, rc, ac, e_e, i_e, j_e, k_e
            )
    LAST_RUN_S = _time.time() - _t0
    LAST_RESULTS = res
    outT = np.concatenate([res.results[dev]["y"] for dev in range(NCORES)], axis=1)
    out = np.ascontiguousarray(outT.T.astype(np.float32))

    # Masked (k==j) triplets: xd is per-edge and xc has no poison encoding,
    # so those triplets contributed silu(W1a^T f_ij + W1c^T f_cos0 + b1)@W2
    # on device (dik features are 0 via fp16 poison; cos enc=0 gives the
    # constant feature vector exp(-g*ac^2)). Subtract that exactly.
    t_bad = np.nonzero(~mask)[0]
    if t_bad.size:
        e_bad = t_bad // D_IN
        d_bad = xd[0, e_bad].astype(np.float32)
        f_ij = np.exp(-GAMMA * (d_bad[:, None] - rc[None, :]) ** 2)
        f_cos0 = np.exp(-GAMMA * ac * ac).astype(np.float32)
        hpre = f_ij @ W1[:K_R] + f_cos0 @ W1[2 * K_R :] + b1
        hb = hpre / (1.0 + np.exp(-hpre))
        np.subtract.at(out, e_bad, (hb @ W2).astype(np.float32))

    if b2.any():
        cnt = np.bincount(e_e, weights=mask.astype(np.float64), minlength=E)
        out = out + cnt[:, None].astype(np.float32) * b2[None, :]
    return out


# revision 39
# speedup vs baseline: 4.8993x; 4.8993x over previous
import sys
from contextlib import ExitStack

import numpy as np

sys.path.insert(0, "/opt/trn_rl_repo")

import jax

# Persistent compilation cache: warm calls skip the per-call NEFF/walrus
# recompile inside the neuronx_cc hook (the executable is cached on disk
# keyed by HLO, which is identical across calls).
try:
    jax.config.update("jax_compilation_cache_dir", "/tmp/bass_jax_cache")
    jax.config.update("jax_persistent_cache_min_compile_time_secs", 0.0)
    jax.config.update("jax_persistent_cache_min_entry_size_bytes", 0)
except Exception:
    pass

import concourse.bass as bass
import concourse.tile as tile
from concourse import bacc, mybir
from concourse.bass_utils import run_bass_kernel_spmd

# Problem constants (hardcoded per harness contract)
N = 10000
D_IN = 12
E = N * D_IN            # 120000 edges
T = E * D_IN            # 1440000 triplets
K_R = 16
K_A = 8
HID = 64
OUT_D = 32
IN_DIM = 2 * K_R + K_A  # 40
N24 = K_R + K_A         # 24 per-triplet (dik/cos) features
GAMMA = 8.0             # same gamma for radial and angular RBFs
EPS = 1e-8
POISON = 30.0           # exp(-8*(30-c)^2) == 0 in f32; fits fp16

NCORES = 8
TD = T // NCORES        # 180000 triplets per core
ED = E // NCORES        # 15000 edges per core
TT = 504                # triplets per tile = 42 edges * 12

# params packing offsets (flat f32 tensor)
P_C16 = 0               # [16,1] rc
P_KAK = 16              # [1,24] 2*g*rc on dik features, else 0
P_KBK = 40              # [1,24] -g on dik features, else 0
P_KAC = 64              # [1,24] (2*g/127)*ac on cos features, else 0
P_KBC = 88              # [1,24] -g/(127*127) on cos features, else 0
P_B24 = 112             # [24,1] -g*c^2
P_W1A = 136             # [16,64] W1 rows 0..16 (dij features)
P_W1B = 1160            # [24,64] W1 rows 16..40
P_B1 = 2696             # [64,1]
P_W2 = 2760             # [64,32]
P_TOT = 4808

F32 = mybir.dt.float32
F16 = mybir.dt.float16
I8 = mybir.dt.int8
QF = 126.0              # int8 quant factor (<127 so fp16 scale rounding
                        # can never push a value past the int8 range)
QCLAMP = 2e-3           # abs-max floor: keeps 126/max inside fp16 range

_PROG = None
LAST_RESULTS = None
LAST_RUN_S = None


def _build_program():
    nc = bacc.Bacc(
        "TRN2", target_bir_lowering=False, debug=False, num_devices=NCORES
    )
    # xd: per-edge distance (fp16, unpoisoned); the dij RBF block is
    # per-edge and broadcast over the 12 triplets of each edge on device.
    XD = nc.dram_tensor("xd", [1, ED], F16, kind="ExternalInput").ap()
    # xk: dik per triplet (fp16, poisoned where k==j)
    XK = nc.dram_tensor("xk", [1, TD], F16, kind="ExternalInput").ap()
    # xc: cos per triplet quantized to int8 (enc = round(cos*127); the
    # decode scale 1/127 is folded into the kaC/kbC stationaries)
    XC = nc.dram_tensor("xc", [1, TD], I8, kind="ExternalInput").ap()
    PRM = nc.dram_tensor("params", [P_TOT], F32, kind="ExternalInput").ap()
    # y8[f,e] = round(out[f,e] * QF / ysc[e]); ysc = per-edge abs-max
    Y8 = nc.dram_tensor("y8", [OUT_D, ED], I8, kind="ExternalOutput").ap()
    YS = nc.dram_tensor("ysc", [1, ED], F16, kind="ExternalOutput").ap()

    with tile.TileContext(nc) as tc, ExitStack() as ctx:
        consts = ctx.enter_context(tc.tile_pool(name="consts", bufs=1))
        inp = ctx.enter_context(tc.tile_pool(name="inp", bufs=4))
        mid = ctx.enter_context(tc.tile_pool(name="mid", bufs=3))
        hp = ctx.enter_context(tc.tile_pool(name="hp", bufs=3))
        psa = ctx.enter_context(
            tc.tile_pool(name="psa", bufs=2, space=bass.MemorySpace.PSUM)
        )
        ps0 = ctx.enter_context(
            tc.tile_pool(name="ps0", bufs=2, space=bass.MemorySpace.PSUM)
        )
        ps1 = ctx.enter_context(
            tc.tile_pool(name="ps1", bufs=2, space=bass.MemorySpace.PSUM)
        )
        ps2 = ctx.enter_context(
            tc.tile_pool(name="ps2", bufs=2, space=bass.MemorySpace.PSUM)
        )

        c16t = consts.tile([K_R, 1], F32)
        nc.gpsimd.dma_start(
            c16t[:], PRM[P_C16 : P_C16 + 16].rearrange("(p f) -> p f", p=16)
        )
        kak = consts.tile([1, N24], F32)
        nc.gpsimd.dma_start(kak[:], PRM[P_KAK : P_KAK + 24].unsqueeze(0))
        kbk = consts.tile([1, N24], F32)
        nc.gpsimd.dma_start(kbk[:], PRM[P_KBK : P_KBK + 24].unsqueeze(0))
        kac = consts.tile([1, N24], F32)
        nc.gpsimd.dma_start(kac[:], PRM[P_KAC : P_KAC + 24].unsqueeze(0))
        kbc = consts.tile([1, N24], F32)
        nc.gpsimd.dma_start(kbc[:], PRM[P_KBC : P_KBC + 24].unsqueeze(0))
        b24t = consts.tile([N24, 1], F32)
        nc.gpsimd.dma_start(
            b24t[:], PRM[P_B24 : P_B24 + 24].rearrange("(p f) -> p f", p=24)
        )
        w1at = consts.tile([K_R, HID], F32)
        nc.gpsimd.dma_start(
            w1at[:], PRM[P_W1A : P_W1A + 1024].rearrange("(p f) -> p f", p=16)
        )
        w1bt = consts.tile([N24, HID], F32)
        nc.gpsimd.dma_start(
            w1bt[:], PRM[P_W1B : P_W1B + 1536].rearrange("(p f) -> p f", p=24)
        )
        b1t = consts.tile([HID, 1], F32)
        nc.gpsimd.dma_start(
            b1t[:], PRM[P_B1 : P_B1 + 64].rearrange("(p f) -> p f", p=64)
        )
        w2t = consts.tile([HID, OUT_D], F32)
        nc.gpsimd.dma_start(
            w2t[:], PRM[P_W2 : P_W2 + 2048].rearrange("(p f) -> p f", p=64)
        )
        out_sb = consts.tile([OUT_D, ED], F32)

        G = TT // D_IN  # edges per tile

        def emit_tile(t0, e0, tt, g):
            """One tile of `tt` triplets / `g` edges; t0/e0 may be symbolic."""
            # --- per-edge dij RBF block -> W1a contribution [HID, g] ---
            dbc = inp.tile([K_R, g], F16)
            nc.gpsimd.dma_start(
                dbc[:], XD[:, bass.ds(e0, g)].partition_broadcast(K_R)
            )
            dsub = mid.tile([K_R, g], F32)
            nc.vector.tensor_scalar_sub(dsub[:], dbc[:], c16t[:])
            dsq = mid.tile([K_R, g], F32)
            nc.vector.tensor_mul(dsq[:], dsub[:], dsub[:])
            fij = mid.tile([K_R, g], F32)
            nc.scalar.activation(
                fij[:], dsq[:], mybir.ActivationFunctionType.Exp, scale=-GAMMA
            )
            pa = psa.tile([HID, g], F32)
            nc.tensor.matmul(pa[:], w1at[:], fij[:])
            ha = hp.tile([HID, g], F32)
            nc.scalar.copy(ha[:], pa[:])

            # --- per-triplet dik/cos features -> W1b contribution [HID, tt] ---
            xkt = inp.tile([1, tt], F16)
            nc.gpsimd.dma_start(xkt[:], XK[:, bass.ds(t0, tt)])
            xct = inp.tile([1, tt], I8)
            nc.gpsimd.dma_start(xct[:], XC[:, bass.ds(t0, tt)])
            xkf = mid.tile([1, tt], F32)
            nc.vector.tensor_copy(xkf[:], xkt[:])
            xk2 = mid.tile([1, tt], F32)
            nc.vector.tensor_mul(xk2[:], xkt[:], xkt[:])
            xcf = mid.tile([1, tt], F32)
            nc.vector.tensor_copy(xcf[:], xct[:])
            xc2 = mid.tile([1, tt], F32)
            nc.vector.tensor_mul(xc2[:], xcf[:], xcf[:])
            p0 = ps0.tile([N24, tt], F32)
            nc.tensor.matmul(p0[:], kak[:], xkf[:], start=True, stop=False)
            nc.tensor.matmul(p0[:], kbk[:], xk2[:], start=False, stop=False)
            nc.tensor.matmul(p0[:], kac[:], xcf[:], start=False, stop=False)
            nc.tensor.matmul(p0[:], kbc[:], xc2[:], start=False, stop=True)
            ft2 = mid.tile([N24, tt], F32)
            nc.scalar.activation(
                ft2[:], p0[:], mybir.ActivationFunctionType.Exp, bias=b24t[:]
            )
            p1 = ps1.tile([HID, tt], F32)
            nc.tensor.matmul(p1[:], w1bt[:], ft2[:])

            # --- combine (broadcast per-edge term over 12 triplets) + MLP ---
            hs = hp.tile([HID, tt], F32)
            nc.vector.tensor_add(
                hs[:].rearrange("p (g s) -> p g s", s=D_IN),
                p1[:].rearrange("p (g s) -> p g s", s=D_IN),
                ha[:].unsqueeze(2).broadcast_to([HID, g, D_IN]),
            )
            h = hp.tile([HID, tt], F32)
            nc.scalar.activation(
                h[:], hs[:], mybir.ActivationFunctionType.Silu, bias=b1t[:]
            )
            p2 = ps2.tile([OUT_D, tt], F32)
            nc.tensor.matmul(p2[:], w2t[:], h[:])

            nc.vector.tensor_reduce(
                out_sb[:, bass.ds(e0, g)],
                p2[:].rearrange("p (g s) -> p g s", s=D_IN),
                axis=mybir.AxisListType.X,
                op=mybir.AluOpType.add,
            )

        nt_full = TD // TT
        tc.For_i_unrolled(
            0,
            nt_full,
            1,
            lambda iv: emit_tile(iv * TT, iv * (TT // D_IN), TT, TT // D_IN),
            max_unroll=8,
        )
        rem = TD - nt_full * TT
        if rem:
            emit_tile(nt_full * TT, nt_full * G, rem, rem // D_IN)

        # int8 quantization with per-edge scale: abs-max over the 32
        # features of each edge, q = out * (QF / max)
        with nc.allow_low_precision("int8 output quant: fp16 scales are ok"):
            mx16 = consts.tile([1, ED], F16)
            nc.gpsimd.tensor_reduce(
                mx16[:],
                out_sb[:],
                axis=mybir.AxisListType.C,
                op=mybir.AluOpType.max,
                apply_absolute_value=True,
            )
            nc.vector.tensor_scalar_max(mx16[:], mx16[:], QCLAMP)
            ri16 = consts.tile([1, ED], F16)
            nc.vector.reciprocal(ri16[:], mx16[:])
            nc.vector.tensor_scalar_mul(ri16[:], ri16[:], QF)
            y8t = consts.tile([OUT_D, ED], I8)
            ones32 = consts.tile([1, OUT_D], F16)
            nc.vector.memset(ones32[:], 1.0)
            # broadcast ri over the 32 partitions via ones^T @ ri chunks
            CH = 500
            for c in range(ED // CH):
                sl = slice(c * CH, (c + 1) * CH)
                pb = ps2.tile([OUT_D, CH], F32)
                nc.tensor.matmul(pb[:], ones32[:], ri16[0:1, sl])
                nc.vector.tensor_mul(out_sb[:, sl], out_sb[:, sl], pb[:])
                nc.vector.tensor_copy(y8t[:, sl], out_sb[:, sl])
            nc.gpsimd.dma_start(Y8[:], y8t[:])
            nc.gpsimd.dma_start(YS[:], mx16[:])

    nc.compile()
    return nc


def _get_program():
    global _PROG
    if _PROG is None:
        _PROG = _build_program()
    return _PROG


def _numpy_fallback(pos, W1, b1, W2, b2, rc, ac, e_e, i_e, j_e, k_e):
    rij = pos[j_e] - pos[i_e]
    rik = pos[k_e] - pos[i_e]
    dij = np.sqrt((rij * rij).sum(-1))
    dik = np.sqrt((rik * rik).sum(-1))
    cos = np.clip((rij * rik).sum(-1) / (dij * dik + EPS), -1.0, 1.0)
    feat = np.concatenate(
        [
            np.exp(-GAMMA * (dij[:, None] - rc[None, :]) ** 2),
            np.exp(-GAMMA * (dik[:, None] - rc[None, :]) ** 2),
            np.exp(-GAMMA * (cos[:, None] - ac[None, :]) ** 2),
        ],
        axis=-1,
    ).astype(np.float32)
    hpre = feat @ W1 + b1
    h = hpre / (1.0 + np.exp(-hpre))
    emb = h @ W2 + b2
    emb *= (k_e != j_e)[:, None].astype(np.float32)
    out = np.zeros((E, OUT_D), np.float32)
    np.add.at(out, e_e, emb)
    return out


def kernel(**inputs) -> np.ndarray:
    global LAST_RESULTS
    pos = np.asarray(inputs["pos"], np.float32)
    W1 = np.asarray(inputs["W1"], np.float32)
    b1 = np.asarray(inputs["b1"], np.float32)
    W2 = np.asarray(inputs["W2"], np.float32)
    b2 = np.asarray(inputs["b2"], np.float32)
    rc = np.asarray(inputs["r_centers"], np.float32)
    ac = np.asarray(inputs["a_centers"], np.float32)
    e_e = np.asarray(inputs["e_e"])
    i_e = np.asarray(inputs["i_e"])
    j_e = np.asarray(inputs["j_e"])
    k_e = np.asarray(inputs["k_e"])

    row = i_e[::D_IN].astype(np.int64)          # source node of each edge
    kidx = (row[:, None] * D_IN + np.arange(D_IN)[None, :]).reshape(-1)  # [T]
    structured = (
        np.array_equal(
            e_e, np.repeat(np.arange(E, dtype=np.int64), D_IN).astype(e_e.dtype)
        )
        and np.array_equal(j_e.astype(np.int64), e_e.astype(np.int64) // D_IN)
        and np.array_equal(i_e.astype(np.int64), np.repeat(row, D_IN))
        and np.array_equal(k_e.astype(np.int64), row[kidx])
    )
    if not structured:
        return _numpy_fallback(pos, W1, b1, W2, b2, rc, ac, e_e, i_e, j_e, k_e)

    # Per-edge geometry on host (E values instead of T), then expand to
    # triplets; device handles RBF + MLP + segment sum.
    col = np.repeat(np.arange(N, dtype=np.int64), D_IN)
    dvec = pos[col] - pos[row]                  # [E,3]
    d = np.sqrt((dvec * dvec).sum(-1))          # [E]
    u = dvec / np.maximum(d, 1e-30)[:, None]    # [E,3] unit vectors

    dik = d[kidx]                               # [T]
    # edge kidx points k->i, so rik = pos[k]-pos[i] = -dvec[kidx]
    cos = np.clip(
        -np.einsum("ts,ts->t", np.repeat(u, D_IN, axis=0), u[kidx]), -1.0, 1.0
    )
    mask = k_e != j_e

    xd = d.astype(np.float16).reshape(1, E)
    xk = np.where(mask, dik, POISON).astype(np.float16).reshape(1, T)
    # masked triplets get enc=0; their (constant) cos-feature contribution is
    # subtracted on the host below
    xc = np.where(mask, np.round(cos * 127.0), 0.0).astype(np.int8).reshape(1, T)

    # dik/cos features: exp(-g*(x-c)^2) = exp(-g*x^2 + 2*g*c*x - g*c^2);
    # for cos the int8 decode scale 1/127 is folded into the coefficients.
    cf24 = np.concatenate([rc, ac]).astype(np.float32)           # [24]
    prm = np.zeros(P_TOT, np.float32)
    prm[P_C16 : P_C16 + 16] = rc
    prm[P_KAK : P_KAK + K_R] = 2.0 * GAMMA * rc
    prm[P_KBK : P_KBK + K_R] = -GAMMA
    prm[P_KAC + K_R : P_KAC + 24] = (2.0 * GAMMA / 127.0) * ac
    prm[P_KBC + K_R : P_KBC + 24] = -GAMMA / (127.0 * 127.0)
    prm[P_B24 : P_B24 + 24] = -GAMMA * cf24 * cf24
    prm[P_W1A : P_W1A + 1024] = W1[:K_R].reshape(-1)
    prm[P_W1B : P_W1B + 1536] = W1[K_R:].reshape(-1)
    prm[P_B1 : P_B1 + 64] = b1
    prm[P_W2 : P_W2 + 2048] = W2.reshape(-1)

    in_maps = []
    for dev in range(NCORES):
        in_maps.append(
            {
                "xd": np.ascontiguousarray(xd[:, dev * ED : (dev + 1) * ED]),
                "xk": np.ascontiguousarray(xk[:, dev * TD : (dev + 1) * TD]),
                "xc": np.ascontiguousarray(xc[:, dev * TD : (dev + 1) * TD]),
                "params": prm,
            }
        )

    import time as _time

    global LAST_RUN_S
    _t0 = _time.time()
    try:
        res = run_bass_kernel_spmd(_get_program(), in_maps, list(range(NCORES)))
    except Exception:
        # transient device errors (NRT_EXEC_UNIT_UNRECOVERABLE) recover on
        # retry; if not, fall back to the (slow but correct) host path
        try:
            res = run_bass_kernel_spmd(
                _get_program(), in_maps, list(range(NCORES))
            )
        except Exception:
            LAST_RUN_S = _time.time() - _t0
            return _numpy_fallback(
                pos, W1, b1, W2, b2, rc, ac, e_e, i_e, j_e, k_e
            )
    LAST_RUN_S = _time.time() - _t0
    LAST_RESULTS = res
    q8 = np.concatenate([res.results[dev]["y8"] for dev in range(NCORES)], axis=1)
    ysc = np.concatenate(
        [res.results[dev]["ysc"] for dev in range(NCORES)], axis=1
    )
    outT = q8.astype(np.float32) * (ysc.astype(np.float32) / QF)
    out = np.ascontiguousarray(outT.T)

    # Masked (k==j) triplets: xd is per-edge and xc has no poison encoding,
    # so those triplets contributed silu(W1a^T f_ij + W1c^T f_cos0 + b1)@W2
    # on device (dik features are 0 via fp16 poison; cos enc=0 gives the
    # constant feature vector exp(-g*ac^2)). Subtract that exactly.
    t_bad = np.nonzero(~mask)[0]
    if t_bad.size:
        e_bad = t_bad // D_IN
        d_bad = xd[0, e_bad].astype(np.float32)
        f_ij = np.exp(-GAMMA * (d_bad[:, None] - rc[None, :]) ** 2)
        f_cos0 = np.exp(-GAMMA * ac * ac).astype(np.float32)
        hpre = f_ij @ W1[:K_R] + f_cos0 @ W1[2 * K_R :] + b1
        hb = hpre / (1.0 + np.exp(-hpre))
        np.subtract.at(out, e_bad, (hb @ W2).astype(np.float32))

    if b2.any():
        cnt = np.bincount(e_e, weights=mask.astype(np.float64), minlength=E)
        out = out + cnt[:, None].astype(np.float32) * b2[None, :]
    return out
